# revision 48
# baseline (speedup 1.0000x reference)
"""AttentiveGraphConvolution (GAT-style layer) on 8 trn2 NeuronCores.

Math (reference):
    h   = x @ W                       [N, D]
    a_s = h @ attn_self               [N, 1]
    a_n = h @ attn_neigh              [N, 1]
    e   = leaky_relu(a_s + a_n.T, 0.2)
    e   = e + NEG_INF * (1 - adj)
    out = relu(softmax(e, -1) @ h)

Reformulation (exact in exact arithmetic):
    exp(leaky(s)) = max(exp(s), exp(0.2 s)),  s_ij = a_s_i + a_n_j.
    Divide softmax numerator and denominator by exp(0.2*a_s_i) and
    exp(0.8*a_s_i) (>0, constant per output row) -- the ratio is unchanged:
        q_ji   = adjT_ji * max(w2_j, invw_i)
        out_i  = relu( (sum_j q_ji v2_j h_j) / (sum_j q_ji v2_j) )
    with w2_j = e^{0.8 a_n_j}, invw_i = e^{-0.8 a_s_i}, v2_j = e^{0.2 a_n_j}.

No collectives: every core recomputes h and a_n for ALL nodes from a bf16
copy of x (cheaper than the ~50 us cross-core barrier + AllGather latency).
Each core's inputs are "rolled" so its own output slab comes first; the
emitted program is identical across cores, only data differs.

Large DMA streams (x blocks, adjacency groups) alternate between the two
hardware DMA queues (SP + Activation) since a single queue sustains only
~200 GB/s.  The a_n row->column transpose bounces through DRAM per 512-node
block, pipelined one block behind compute so the round-trip latency hides.

Device main loop per 128-node j-chunk (64 chunks):
    h2w = v2_chunk * h_chunk                    -- ACT scale-copy (idle engine)
    q   = (invw_bcast MAX w2col) * adjT_chunk   -- one DVE STT, all bf16
    numT += h2w.T @ q                           -- PE bf16
    den  += v2col.T @ q                         -- PE bf16
Output is produced transposed [dout, s]; the host transposes back.
"""

import numpy as np

N = 8192
DIN = 512
DOUT = 128
NCORES = 8
S = N // NCORES     # 1024 output rows per core
GP = 4              # adjacency rows per partition per DMA block
NBX = 512           # x block size (nodes per phase-1 block)


def _emit(nc, tc, ctx, n, s, din, dout):
    from concourse import masks, mybir

    f32 = mybir.dt.float32
    bf16 = mybir.dt.bfloat16
    fp8 = mybir.dt.float8e4
    AF = mybir.ActivationFunctionType
    ALU = mybir.AluOpType

    P = 128
    jc_n = n // P           # 64 j-chunks over all nodes
    kc_n = din // P         # 4 contraction chunks for x @ W
    nb = min(512, s)        # matmul moving-dim block (PSUM bank limit)
    ib_n = s // nb          # i blocks per core
    xb_n = n // NBX         # 16 x blocks
    cb_n = NBX // P         # 4 chunks per x block

    adjt = nc.dram_tensor("adjt", [n, s], fp8, kind="ExternalInput")
    xt = nc.dram_tensor("xt", [xb_n * P, kc_n * NBX], bf16, kind="ExternalInput")
    wmat = nc.dram_tensor("wmat", [din, dout], bf16, kind="ExternalInput")
    att = nc.dram_tensor("att", [dout, 2], bf16, kind="ExternalInput")
    out = nc.dram_tensor("out", [dout, s], f32, kind="ExternalOutput")

    const_pool = ctx.enter_context(tc.tile_pool(name="const", bufs=1))
    ph1_pool = ctx.enter_context(tc.tile_pool(name="ph1", bufs=1))
    xb_pool = ctx.enter_context(tc.tile_pool(name="xb", bufs=4))
    ph1_psum = ctx.enter_context(tc.tile_pool(name="ph1_psum", bufs=2, space="PSUM"))
    tp_psum = ctx.enter_context(tc.tile_pool(name="tp_psum", bufs=1, space="PSUM"))
    hn_psum = ctx.enter_context(tc.tile_pool(name="hn_psum", bufs=1, space="PSUM"))
    acc_psum = ctx.enter_context(tc.tile_pool(name="acc_psum", bufs=1, space="PSUM"))
    dram_pool = ctx.enter_context(tc.tile_pool(name="dram", bufs=1, space="DRAM"))
    adj_pool = ctx.enter_context(tc.tile_pool(name="adj", bufs=6))
    q_pool = ctx.enter_context(tc.tile_pool(name="q", bufs=6))

    ident = const_pool.tile([P, P], f32, name="ident")
    masks.make_identity(nc, ident[:])
    ident_bf = const_pool.tile([P, P], bf16, name="ident_bf")
    nc.scalar.activation(ident_bf[:], ident[:], AF.Copy)
    ones_bf = const_pool.tile([1, P], bf16, name="ones_bf")
    nc.gpsimd.memset(ones_bf[:], 1.0)

    w_sb = []
    for k in range(kc_n):
        wt = ph1_pool.tile([P, P], bf16, name="w_sb", tag=f"w_sb{k}")
        nc.scalar.dma_start(wt[:], wmat[k * P:(k + 1) * P, :])
        w_sb.append(wt)
    att_sb = const_pool.tile([P, 2], bf16, name="att_sb")
    nc.scalar.dma_start(att_sb[:], att[:])

    # ---- Phase 1: h for ALL nodes, a_s/a_n, transposed h chunks ------------
    hT_sb = ph1_pool.tile([P, n], bf16, name="hT_sb")     # h^T, all nodes
    av_sb = ph1_pool.tile([2, n], f32, name="av_sb")      # a_s, a_n rows
    h2raw = ph1_pool.tile([P, n], bf16, name="h2raw")     # h chunks, [j, d]
    an_dram = dram_pool.tile([n, 1], f32, name="an_dram")
    anf_pool = ctx.enter_context(tc.tile_pool(name="anf", bufs=4))
    anf_raw = [None] * xb_n
    w2col = ph1_pool.tile([P, jc_n], f32, name="w2col")
    v2col = ph1_pool.tile([P, jc_n], f32, name="v2col")
    v2colbf = ph1_pool.tile([P, jc_n], bf16, name="v2colbf")
    invw_sb = const_pool.tile([P, s], bf16, name="invw_sb")

    def an_finish(b):
        # anf_raw[b] [4, 128] -> [128, 4] column transpose, then exps
        anT_ps = tp_psum.tile([P, cb_n], f32, name="anT_ps", tag="tp")
        nc.tensor.matmul(anT_ps[:], anf_raw[b][:], ident[:cb_n, :cb_n],
                         is_transpose=True, start=True, stop=True)
        cs = slice(cb_n * b, cb_n * (b + 1))
        nc.scalar.activation(w2col[:, cs], anT_ps[:], AF.Exp, scale=0.8)
        nc.scalar.activation(v2col[:, cs], anT_ps[:], AF.Exp, scale=0.2)
        nc.scalar.activation(v2colbf[:, cs], anT_ps[:], AF.Exp, scale=0.2)
        # transpose h chunks of block b, then v2-scaled copy to h2raw
        hnb = hn_psum.tile([P, cb_n * P], bf16, name="hnb")
        for cc in range(cb_n):
            c = b * cb_n + cc
            nc.tensor.matmul(
                hnb[:, cc * P:(cc + 1) * P], hT_sb[:, c * P:(c + 1) * P],
                ident_bf[:], is_transpose=True, start=True, stop=True,
            )
        for cc in range(cb_n):
            c = b * cb_n + cc
            nc.scalar.activation(h2raw[:, c * P:(c + 1) * P],
                                 hnb[:, cc * P:(cc + 1) * P],
                                 AF.Copy, scale=v2col[:, c:c + 1])

    # main-loop accumulators; chunk emission is software-pipelined into the
    # phase-1 block loop (group g emitted at block g+3) so main-loop PE/DVE
    # work overlaps the tail of the x DMA stream
    mm_ps = [acc_psum.tile([P, nb], f32, name=f"mm_ps{b}") for b in range(ib_n)]
    rs_ps = [acc_psum.tile([1, nb], f32, name=f"rs_ps{b}") for b in range(ib_n)]
    adj_cur = [None]

    def emit_chunk(pos):
        r = pos % GP
        if r == 0:
            G = pos // GP
            adj_cur[0] = adj_pool.tile([P, GP * s], fp8, name="adj_t")
            nc.sync.dma_start(
                adj_cur[0][:],
                adjt[G * GP * P:(G + 1) * GP * P, :].rearrange(
                    "(p r) i -> p (r i)", r=GP),
            )
        adj_t = adj_cur[0]
        q_t = q_pool.tile([P, s], bf16, name="q_t")
        nc.vector.scalar_tensor_tensor(
            q_t[:], invw_sb[:], w2col[:, pos:pos + 1],
            adj_t[:, r * s:(r + 1) * s],
            op0=ALU.max, op1=ALU.mult,
        )
        for b in range(ib_n):
            nc.tensor.matmul(
                mm_ps[b][:], h2raw[:, pos * P:(pos + 1) * P],
                q_t[:, b * nb:(b + 1) * nb],
                start=(pos == 0), stop=(pos == jc_n - 1),
            )
        for b in range(ib_n):
            nc.tensor.matmul(
                rs_ps[b][:], v2colbf[:, pos:pos + 1], q_t[:, b * nb:(b + 1) * nb],
                start=(pos == 0), stop=(pos == jc_n - 1),
            )

    for b in range(xb_n):
        xb = xb_pool.tile([P, kc_n * NBX], bf16, name="xb")
        nc.sync.dma_start(xb[:], xt[b * P:(b + 1) * P, :])
        sl = slice(b * NBX, (b + 1) * NBX)
        hT_ps = ph1_psum.tile([P, NBX], f32, name="hT_ps")
        for k in range(kc_n):
            nc.tensor.matmul(
                hT_ps[:], w_sb[k][:], xb[:, k * NBX:(k + 1) * NBX],
                start=(k == 0), stop=(k == kc_n - 1),
            )
        nc.scalar.activation(hT_sb[:, sl], hT_ps[:], AF.Copy)
        av_ps = tp_psum.tile([2, NBX], f32, name="av_ps", tag="tp")
        nc.tensor.matmul(av_ps[:], att_sb[:], hT_sb[:, sl],
                         start=True, stop=True)
        nc.scalar.activation(av_sb[:, sl], av_ps[:], AF.Copy)
        nc.scalar.dma_start(
            an_dram[sl, :].rearrange("s o -> o s"), av_sb[1:2, sl])
        anf_raw[b] = anf_pool.tile([cb_n, P], f32, name="anf_raw")
        nc.scalar.dma_start(
            anf_raw[b][:],
            an_dram[sl, :].rearrange("(k p) o -> k (p o)", p=P))
        if b == 1:
            # invw_bcast[p, i] = exp(-0.8 * a_s_own[i]); own slab = blocks 0-1
            wrow_sb = ph1_pool.tile([1, s], bf16, name="wrow_sb")
            nc.scalar.activation(wrow_sb[:], av_sb[0:1, 0:s], AF.Exp,
                                 scale=-0.8)
            for bb in range(ib_n):
                wb_ps = tp_psum.tile([P, nb], f32, name="wb_ps", tag="tp")
                nc.tensor.matmul(
                    wb_ps[:], ones_bf[:], wrow_sb[:, bb * nb:(bb + 1) * nb],
                    start=True, stop=True,
                )
                nc.scalar.activation(invw_sb[:, bb * nb:(bb + 1) * nb],
                                     wb_ps[:], AF.Copy)
        if b >= 1:
            an_finish(b - 1)
            for pos in range(cb_n * (b - 1), cb_n * b):
                emit_chunk(pos)
    an_finish(xb_n - 1)
    for pos in range(max(0, cb_n * (xb_n - 1)), jc_n):
        emit_chunk(pos)

    # ---- Phase 3: normalize + relu (output stays transposed) ---------------
    rs_sb = ph1_pool.tile([1, s], f32, name="rs_sb")
    for b in range(ib_n):
        nc.scalar.activation(rs_sb[:, b * nb:(b + 1) * nb], rs_ps[b][:], AF.Copy)
    rr_sb = ph1_pool.tile([1, s], f32, name="rr_sb")
    nc.vector.reciprocal_approx_fast(rr_sb[:], rs_sb[:])
    rr_bf = ph1_pool.tile([1, s], bf16, name="rr_bf")
    nc.scalar.activation(rr_bf[:], rr_sb[:], AF.Copy)
    rrb_sb = ph1_pool.tile([P, s], f32, name="rrb_sb")
    for b in range(ib_n):
        rrb_ps = tp_psum.tile([P, nb], f32, name="rrb_ps", tag="tp")
        nc.tensor.matmul(
            rrb_ps[:], ones_bf[:], rr_bf[:, b * nb:(b + 1) * nb],
            start=True, stop=True,
        )
        nc.scalar.activation(rrb_sb[:, b * nb:(b + 1) * nb], rrb_ps[:], AF.Copy)
    out_sb = ph1_pool.tile([P, s], f32, name="out_sb")
    for b in range(ib_n):
        nc.vector.scalar_tensor_tensor(
            out_sb[:, b * nb:(b + 1) * nb], mm_ps[b][:], 0.0,
            rrb_sb[:, b * nb:(b + 1) * nb],
            op0=ALU.max, op1=ALU.mult,
        )
        nc.scalar.dma_start(out[:, b * nb:(b + 1) * nb],
                            out_sb[:, b * nb:(b + 1) * nb])


def build_nc(n=N, s=S, din=DIN, dout=DOUT):
    from contextlib import ExitStack

    import concourse.bacc as bacc
    import concourse.tile as tile

    nc = bacc.Bacc(
        "TRN2",
        target_bir_lowering=False,
        debug=False,
        num_devices=NCORES,
    )
    with tile.TileContext(nc) as tc, ExitStack() as ctx:
        _emit(nc, tc, ctx, n, s, din, dout)
    nc.compile()
    return nc


def prep_adjt(adj_slab_rolled):
    """[s, n] rolled adj row-slab -> transposed [n, s] bf16 with GP-row
    interleave per 512-row block."""
    import ml_dtypes

    adjt = adj_slab_rolled.T  # [n, s]
    n, s = adjt.shape
    P = 128
    g = n // (GP * P)
    adjt = adjt.reshape(g, GP, P, s).transpose(0, 2, 1, 3).reshape(n, s)
    return np.ascontiguousarray(adjt.astype(ml_dtypes.float8_e4m3fn))


def prep_xt(x_rolled):
    """[n, din] rolled x -> [xb_n*128, kc_n*NBX] bf16: block b row p holds
    x[b*NBX + i, k*128 + p] at column k*NBX + i, so each partition reads one
    contiguous 4 KB run per block."""
    import ml_dtypes

    n, din = x_rolled.shape
    P = 128
    kc = din // P
    blocks = []
    for b in range(n // NBX):
        blk = x_rolled[b * NBX:(b + 1) * NBX, :].T  # [din, NBX]
        blk = blk.reshape(kc, P, NBX).transpose(1, 0, 2).reshape(P, kc * NBX)
        blocks.append(blk)
    return np.ascontiguousarray(
        np.concatenate(blocks, axis=0).astype(ml_dtypes.bfloat16))


def make_in_maps(x, adj, W, attn_self, attn_neigh, s=S):
    import ml_dtypes

    att = np.concatenate([attn_self, attn_neigh], axis=1)
    att_bf = np.ascontiguousarray(att.astype(ml_dtypes.bfloat16))
    w_bf = np.ascontiguousarray(W.astype(ml_dtypes.bfloat16))
    in_maps = []
    for c in range(NCORES):
        sl = slice(c * s, (c + 1) * s)
        roll = np.roll(np.arange(N), -c * s)
        in_maps.append({
            "adjt": prep_adjt(adj[sl, :][:, roll]),
            "xt": prep_xt(x[roll, :]),
            "wmat": w_bf,
            "att": att_bf,
        })
    return in_maps


def kernel(x, adj, W, attn_self, attn_neigh):
    from concourse.bass_utils import run_bass_kernel_spmd

    x = np.asarray(x, dtype=np.float32)
    adj = np.asarray(adj, dtype=np.float32)
    W = np.asarray(W, dtype=np.float32)
    attn_self = np.asarray(attn_self, dtype=np.float32)
    attn_neigh = np.asarray(attn_neigh, dtype=np.float32)

    nc = build_nc()
    in_maps = make_in_maps(x, adj, W, attn_self, attn_neigh)
    res = run_bass_kernel_spmd(nc, in_maps, list(range(NCORES)))
    return np.concatenate(
        [np.ascontiguousarray(res.results[c]["out"].T) for c in range(NCORES)],
        axis=0)
